# revision 1
# baseline (speedup 1.0000x reference)
"""Chamfer distance loss kernel for 8x trn2 NeuronCores.

pred/target: [8, 4096, 3] f32. Output: scalar f32 (shape ()).

Sharding: data-parallel over batch, 1 batch per core; host sums the
per-core partial min-sums (no collectives needed).

Per core (N = M = 4096 points, d = 3):
  1. Host builds K=18 bf16 augmented matrices A, B such that
     D[n, m] = sum_k A[k, n] * B[k, m] equals the squared pairwise
     distance |p_n - t_m|^2 to ~1e-7 abs (hi/lo bf16 product splits +
     3-term splits of the squared norms). bf16 matmuls run ~3.4x
     faster than fp32 on the PE.
  2. PE: per 128-row n-tile, 8 x 512-wide matmuls packed 4-way with
     tile_position row groups (K=18 <= 32 rows) -> [128, 4096] distance
     tile in PSUM (fp32) at ~4x the unpacked rate.
  3. ACT (ScalarE) stages each PSUM tile to SBUF as fp16 (the only
     engine besides DVE that can read PSUM; frees DVE for min work).
  4. DVE (VectorE, the only min-capable engine): row-min via two
     halving tensor_tensor mins + short reduce (16-bit 2x mode);
     col-min via a running elementwise min fold into a [128, 4096]
     accumulator (copy-init on the first tile).
  5. Col-min's partition-axis reduction: PE transposes of the
     accumulator + strided DVE reduces; [128, 32] row/col min tensors
     are DMA'd out and summed on host.
"""

import sys

import numpy as np

for _p in ("/opt/trn_rl_repo",):
    if _p not in sys.path:
        sys.path.insert(0, _p)

import concourse.bass as bass
import concourse.mybir as mybir
import concourse.tile as tile
from concourse import bacc, bass2jax
from concourse.masks import make_identity

B = 8
NPTS = 4096
K_AUG = 5
K_SPLIT = 13
K_SPLIT2 = 18
P = 128
N_TILES = NPTS // P  # 32
MG = 2048  # m-group width (4 PSUM banks)
N_GROUPS = NPTS // MG  # 2
MM_N = 512  # free dim per matmul (1 PSUM bank)
BIG = 3.0e38

_cached = {}


def build_nc_v5(repeat=1, variant="v5"):
    """bf16-split K=13 matmuls, 4x row-group packed; ACT stages into one
    [128, 4096] bf16 tile per n-tile; DVE: 1 fold + 2 halves + 1 reduce."""
    f32 = mybir.dt.float32
    bf16 = mybir.dt.bfloat16
    MIN = mybir.AluOpType.min
    AX = mybir.AxisListType.X
    NQ = 4  # row groups
    k_split = K_SPLIT2 if "x" in variant else K_SPLIT
    sdt = mybir.dt.float16 if "f16" in variant else bf16
    big = 6.0e4 if "f16" in variant else BIG
    nc = bacc.Bacc("TRN2", target_bir_lowering=False, debug=False, num_devices=B)

    a_dram = nc.dram_tensor("a", [k_split, NPTS], bf16, kind="ExternalInput")
    b_dram = nc.dram_tensor("b", [k_split, NPTS], bf16, kind="ExternalInput")
    rowmins_dram = nc.dram_tensor("rowmins", [P, N_TILES], f32, kind="ExternalOutput")
    colmins_dram = nc.dram_tensor("colmins", [P, N_TILES], f32, kind="ExternalOutput")

    with tile.TileContext(nc) as tc:
        with (
            tc.tile_pool(name="const", bufs=1) as cpool,
            tc.tile_pool(name="acc", bufs=1) as apool,
            tc.tile_pool(name="stage", bufs=(7 if "b7" in variant else 5 if "b5" in variant else 3)) as spool,
            tc.tile_pool(name="scr", bufs=3) as scrpool,
            tc.tile_pool(name="psum", bufs=2, space=bass.MemorySpace.PSUM) as ppool,
        ):
            a4 = cpool.tile([96 + k_split, NPTS], bf16)
            b4 = cpool.tile([96 + k_split, NPTS], bf16)
            ident16 = cpool.tile([P, P], sdt)
            for q in range(NQ):
                nc.sync.dma_start(a4[32 * q : 32 * q + k_split, :], a_dram[:])
                nc.sync.dma_start(b4[32 * q : 32 * q + k_split, :], b_dram[:])
            make_identity(nc, ident16[:])

            cacc16 = apool.tile([P, NPTS], sdt)
            scratch_shared = apool.tile([P, MG], sdt)
            rowmins_sb = apool.tile([P, N_TILES], f32)
            colmins_sb = apool.tile([P, N_TILES], f32)

            for _rep in range(repeat):
                for t in range(N_TILES):
                    tsl = slice(t * P, (t + 1) * P)
                    pts = [
                        ppool.tile([P, MG], f32, tag="ptile", name=f"pt{g}_{t}")
                        for g in range(N_GROUPS)
                    ]
                    for step in range(2):
                        for q in range(NQ):
                            c = step * NQ + q  # m-chunk 0..7
                            g, j = divmod(c, MG // MM_N)
                            nc.tensor.matmul(
                                pts[g][:, j * MM_N : (j + 1) * MM_N],
                                a4[32 * q : 32 * q + k_split, tsl],
                                b4[32 * q : 32 * q + k_split, c * MM_N : (c + 1) * MM_N],
                                start=True,
                                stop=True,
                                tile_position=(32 * q, 0),
                            )
                    half_stage = "hs" in variant
                    st_w = MG if half_stage else NPTS
                    st = spool.tile([P, st_w], sdt, tag="stage", name=f"st_{t}")
                    if "sc" in variant:
                        scratch = scrpool.tile([P, MG], sdt, tag="scr", name=f"scr_{t}")
                    else:
                        scratch = scratch_shared
                    if "mm" not in variant:
                        if half_stage:
                            nc.scalar.copy(st[:], pts[0][:])
                        else:
                            for g in range(N_GROUPS):
                                nc.scalar.copy(st[:, g * MG : (g + 1) * MG], pts[g][:])
                    if "es" in variant and "mm" not in variant and "so" not in variant:
                        # per-group ops: DVE starts after the first ACT copy
                        for g in range(N_GROUPS):
                            gs = st[:, g * MG : (g + 1) * MG]
                            nc.vector.tensor_tensor(
                                out=scratch[:, g * (MG // 2) : (g + 1) * (MG // 2)],
                                in0=st[:, g * MG : g * MG + MG // 2],
                                in1=st[:, g * MG + MG // 2 : (g + 1) * MG],
                                op=MIN,
                            )
                            if t == 0:
                                nc.vector.tensor_copy(
                                    cacc16[:, g * MG : (g + 1) * MG], gs
                                )
                            else:
                                nc.vector.tensor_tensor(
                                    out=cacc16[:, g * MG : (g + 1) * MG],
                                    in0=gs,
                                    in1=cacc16[:, g * MG : (g + 1) * MG],
                                    op=MIN,
                                )
                        nc.vector.tensor_tensor(
                            out=scratch[:, 0 : MG // 2],
                            in0=scratch[:, 0 : MG // 2],
                            in1=scratch[:, MG // 2 : MG],
                            op=MIN,
                        )
                        nc.vector.tensor_reduce(
                            out=rowmins_sb[:, t : t + 1],
                            in_=scratch[:, 0 : MG // 2],
                            axis=AX,
                            op=MIN,
                        )
                    elif "mm" not in variant and "so" not in variant:
                        if "nd" not in variant:
                            if half_stage:
                                nc.vector.tensor_tensor(
                                    out=scratch[:],
                                    in0=st[:],
                                    in1=pts[1][:],
                                    op=MIN,
                                )
                            else:
                                nc.vector.tensor_tensor(
                                    out=scratch[:],
                                    in0=st[:, 0:MG],
                                    in1=st[:, MG:NPTS],
                                    op=MIN,
                                )
                            nc.vector.tensor_tensor(
                                out=scratch[:, 0 : MG // 2],
                                in0=scratch[:, 0 : MG // 2],
                                in1=scratch[:, MG // 2 : MG],
                                op=MIN,
                            )
                            nc.vector.tensor_reduce(
                                out=rowmins_sb[:, t : t + 1],
                                in_=scratch[:, 0 : MG // 2],
                                axis=AX,
                                op=MIN,
                            )
                        if "nf" not in variant:
                            if half_stage:
                                if t == 0:
                                    nc.vector.tensor_copy(cacc16[:, 0:MG], st[:])
                                    nc.vector.tensor_copy(cacc16[:, MG:NPTS], pts[1][:])
                                else:
                                    nc.vector.tensor_tensor(
                                        out=cacc16[:, 0:MG],
                                        in0=st[:],
                                        in1=cacc16[:, 0:MG],
                                        op=MIN,
                                    )
                                    nc.vector.tensor_tensor(
                                        out=cacc16[:, MG:NPTS],
                                        in0=pts[1][:],
                                        in1=cacc16[:, MG:NPTS],
                                        op=MIN,
                                    )
                            elif t == 0:
                                nc.vector.tensor_copy(cacc16[:], st[:])
                            else:
                                nc.vector.tensor_tensor(
                                    out=cacc16[:], in0=st[:], in1=cacc16[:], op=MIN
                                )

                stripped = any(f in variant for f in ("mm", "so", "nd", "nf"))
                if stripped:
                    nc.gpsimd.memset(rowmins_sb[:], 0.0)
                    if "nf" in variant or "mm" in variant or "so" in variant:
                        nc.gpsimd.memset(cacc16[:], big)
                if "nt" in variant:
                    nc.sync.dma_start(rowmins_dram[:], rowmins_sb[:])
                    nc.sync.dma_start(colmins_dram[:], rowmins_sb[:])
                    continue
                for gg in range(N_TILES // 4):
                    tp = ppool.tile([P, 4, P], sdt, tag="ptile", name=f"tp_{gg}")
                    for j in range(4):
                        c = gg * 4 + j
                        nc.tensor.transpose(
                            tp[:, j, :], cacc16[:, c * P : (c + 1) * P], ident16[:]
                        )
                    nc.vector.tensor_reduce(
                        out=colmins_sb[:, gg * 4 : (gg + 1) * 4],
                        in_=tp[:],
                        axis=AX,
                        op=MIN,
                    )
                nc.sync.dma_start(rowmins_dram[:], rowmins_sb[:])
                nc.sync.dma_start(colmins_dram[:], colmins_sb[:])

    nc.compile()
    return nc


def build_nc(repeat=1, variant="v3"):
    if variant.startswith("v5"):
        return build_nc_v5(repeat=repeat, variant=variant)
    f32 = mybir.dt.float32
    bf16 = mybir.dt.bfloat16
    MIN = mybir.AluOpType.min
    AX = mybir.AxisListType.X
    nc = bacc.Bacc("TRN2", target_bir_lowering=False, debug=False, num_devices=B)

    split16 = variant.startswith("v4")
    k_aug = K_SPLIT if split16 else K_AUG
    in_dt = bf16 if split16 else f32
    a_dram = nc.dram_tensor("a", [k_aug, NPTS], in_dt, kind="ExternalInput")
    b_dram = nc.dram_tensor("b", [k_aug, NPTS], in_dt, kind="ExternalInput")
    rowpart_dram = nc.dram_tensor(
        "rowpart", [P, N_TILES * N_GROUPS], f32, kind="ExternalOutput"
    )
    colmins_dram = nc.dram_tensor("colmins", [P, N_TILES], f32, kind="ExternalOutput")

    with tile.TileContext(nc) as tc:
        with (
            tc.tile_pool(name="const", bufs=1) as cpool,
            tc.tile_pool(name="acc", bufs=1) as apool,
            tc.tile_pool(name="stage", bufs=4) as spool,
            tc.tile_pool(name="psum", bufs=2, space=bass.MemorySpace.PSUM) as ppool,
        ):
            a_sb = cpool.tile([k_aug, NPTS], in_dt)
            b_sb = cpool.tile([k_aug, NPTS], in_dt)
            ident = cpool.tile([P, P], f32)
            nc.sync.dma_start(a_sb[:], a_dram[:])
            nc.sync.dma_start(b_sb[:], b_dram[:])
            make_identity(nc, ident[:])
            if "b16" in variant and not split16:
                a16_sb = cpool.tile([k_aug, NPTS], bf16)
                b16_sb = cpool.tile([k_aug, NPTS], bf16)
                nc.vector.tensor_copy(a16_sb[:], a_sb[:])
                nc.vector.tensor_copy(b16_sb[:], b_sb[:])
                mm_a, mm_b = a16_sb, b16_sb
            else:
                mm_a, mm_b = a_sb, b_sb

            cacc = apool.tile([P, NPTS], f32)  # running col-min accumulator
            cacc16 = apool.tile([P, NPTS], bf16)
            scratch = apool.tile([P, MG // 2], bf16)
            rowpart_sb = apool.tile([P, N_TILES * N_GROUPS], f32)
            colmins_sb = apool.tile([P, N_TILES], f32)

            for _rep in range(repeat):
                if variant == "v1":
                    nc.gpsimd.memset(cacc[:], BIG)
                else:
                    nc.gpsimd.memset(cacc16[:], BIG)
                if any(f in variant for f in ("mm", "so", "nd", "nf")):
                    nc.gpsimd.memset(rowpart_sb[:], 0.0)

                for t in range(N_TILES):
                    lhsT = mm_a[:, t * P : (t + 1) * P]
                    for g in range(N_GROUPS):
                        pt = ppool.tile([P, MG], f32, tag="ptile")
                        for j in range(MG // MM_N):
                            off = g * MG + j * MM_N
                            nc.tensor.matmul(
                                pt[:, j * MM_N : (j + 1) * MM_N],
                                lhsT,
                                mm_b[:, off : off + MM_N],
                                start=True,
                                stop=True,
                            )
                        col = t * N_GROUPS + g
                        if variant == "v1":
                            # all-DVE
                            nc.vector.tensor_tensor(
                                out=cacc[:, g * MG : (g + 1) * MG],
                                in0=pt[:],
                                in1=cacc[:, g * MG : (g + 1) * MG],
                                op=MIN,
                            )
                            nc.vector.tensor_reduce(
                                out=rowpart_sb[:, col : col + 1],
                                in_=pt[:],
                                axis=AX,
                                op=MIN,
                            )
                        else:
                            # v3: ACT stages PSUM->SBUF bf16; DVE works in
                            # bf16 2x mode: TT halve + short reduce for
                            # dir-1, TT fold for the dir-2 accumulator.
                            st = spool.tile([P, MG], bf16, tag="stage")
                            if "mm" not in variant:
                                nc.scalar.copy(st[:], pt[:])
                            if "mm" not in variant and "so" not in variant:
                                if "nd" not in variant:
                                    nc.vector.tensor_tensor(
                                        out=scratch[:],
                                        in0=st[:, 0 : MG // 2],
                                        in1=st[:, MG // 2 : MG],
                                        op=MIN,
                                    )
                                    nc.vector.tensor_reduce(
                                        out=rowpart_sb[:, col : col + 1],
                                        in_=scratch[:],
                                        axis=AX,
                                        op=MIN,
                                    )
                                if "nf" not in variant:
                                    nc.vector.tensor_tensor(
                                        out=cacc16[:, g * MG : (g + 1) * MG],
                                        in0=st[:],
                                        in1=cacc16[:, g * MG : (g + 1) * MG],
                                        op=MIN,
                                    )

                # col-min partition-axis reduction: transpose 128-wide chunks
                # with PE, then strided reduce (innermost axis only).
                if "nt" in variant:
                    nc.sync.dma_start(rowpart_dram[:], rowpart_sb[:])
                    nc.sync.dma_start(colmins_dram[:], rowpart_sb[:, 0:N_TILES])
                    continue
                if variant != "v1":
                    for g in range(N_GROUPS):
                        nc.vector.tensor_copy(
                            cacc[:, g * MG : (g + 1) * MG],
                            cacc16[:, g * MG : (g + 1) * MG],
                        )
                for gg in range(N_TILES // 4):
                    tp = ppool.tile([P, 4, P], f32, tag="ptile")
                    for j in range(4):
                        c = gg * 4 + j
                        nc.tensor.transpose(
                            tp[:, j, :], cacc[:, c * P : (c + 1) * P], ident[:]
                        )
                    nc.vector.tensor_reduce(
                        out=colmins_sb[:, gg * 4 : (gg + 1) * 4],
                        in_=tp[:],
                        axis=AX,
                        op=MIN,
                    )

                nc.sync.dma_start(rowpart_dram[:], rowpart_sb[:])
                nc.sync.dma_start(colmins_dram[:], colmins_sb[:])

    nc.compile()
    return nc


class Runner:
    """Caches the jitted shard_map executable across calls (the stock
    run_bass_kernel_spmd axon path rebuilds it per call, ~300 ms)."""

    def __init__(self, nc, n_cores):
        import jax
        from jax.experimental.shard_map import shard_map
        from jax.sharding import Mesh, PartitionSpec

        bass2jax.install_neuronx_cc_hook()
        self.nc = nc
        self.n_cores = n_cores
        partition_name = (
            nc.partition_id_tensor.name if nc.partition_id_tensor else None
        )
        in_names, out_names, out_avals, zero_outs = [], [], [], []
        for alloc in nc.m.functions[0].allocations:
            if not isinstance(alloc, mybir.MemoryLocationSet):
                continue
            name = alloc.memorylocations[0].name
            if alloc.kind == "ExternalInput":
                if name != partition_name:
                    in_names.append(name)
            elif alloc.kind == "ExternalOutput":
                shape = tuple(alloc.tensor_shape)
                dtype = mybir.dt.np(alloc.dtype)
                out_avals.append(jax.core.ShapedArray(shape, dtype))
                zero_outs.append(np.zeros(shape, dtype))
                out_names.append(name)
        self.in_names = list(in_names)
        self.out_names = out_names
        self.out_avals = out_avals
        self.zero_outs = zero_outs
        n_params = len(in_names)
        all_names = in_names + out_names
        if partition_name is not None:
            all_names = all_names + [partition_name]

        def _body(*args):
            operands = list(args)
            if partition_name is not None:
                operands.append(bass2jax.partition_id_tensor())
            outs = bass2jax._bass_exec_p.bind(
                *operands,
                out_avals=tuple(out_avals),
                in_names=tuple(all_names),
                out_names=tuple(out_names),
                lowering_input_output_aliases=(),
                sim_require_finite=True,
                sim_require_nnan=True,
                nc=nc,
            )
            return tuple(outs)

        devices = jax.devices()[:n_cores]
        mesh = Mesh(np.asarray(devices), ("core",))
        n_outs = len(out_names)
        self._sharded = jax.jit(
            shard_map(
                _body,
                mesh=mesh,
                in_specs=(PartitionSpec("core"),) * (n_params + n_outs),
                out_specs=(PartitionSpec("core"),) * n_outs,
                check_rep=False,
            ),
            donate_argnums=tuple(range(n_params, n_params + n_outs)),
            keep_unused=True,
        )

    def run_raw(self, in_maps):
        """Returns unblocked jax output arrays (call np.asarray to sync)."""
        n = self.n_cores
        concat_in = [
            np.concatenate([in_maps[c][name] for c in range(n)], axis=0)
            for name in self.in_names
        ]
        concat_zeros = [
            np.zeros((n * z.shape[0], *z.shape[1:]), z.dtype) for z in self.zero_outs
        ]
        return self._sharded(*concat_in, *concat_zeros)

    def __call__(self, in_maps):
        out_arrs = self.run_raw(in_maps)
        n = self.n_cores
        return [
            {
                name: np.asarray(out_arrs[i]).reshape(n, *self.out_avals[i].shape)[c]
                for i, name in enumerate(self.out_names)
            }
            for c in range(n)
        ]


def get_runner(repeat=1, variant="v3"):
    key = (repeat, variant)
    if key not in _cached:
        _cached[key] = Runner(build_nc(repeat=repeat, variant=variant), B)
    return _cached[key]


def make_in_maps(pred, target, kind="f32"):
    if kind == "split16":
        return _make_in_maps_split16(pred, target)
    if kind == "split18":
        return _make_in_maps_split18(pred, target)
    in_maps = []
    for c in range(B):
        p = np.ascontiguousarray(pred[c], dtype=np.float32)
        t = np.ascontiguousarray(target[c], dtype=np.float32)
        psq = (p * p).sum(axis=1)
        tsq = (t * t).sum(axis=1)
        a = np.empty((K_AUG, NPTS), dtype=np.float32)
        a[0:3] = -2.0 * p.T
        a[3] = psq
        a[4] = 1.0
        bm = np.empty((K_AUG, NPTS), dtype=np.float32)
        bm[0:3] = t.T
        bm[3] = 1.0
        bm[4] = tsq
        in_maps.append({"a": a, "b": bm})
    return in_maps


def _make_in_maps_split16(pred, target):
    """K=13 bf16 hi/lo-split augmentation.

    D = |p|^2 + |t|^2 - 2 p.t reproduced to ~1e-5 abs via
    x ~ hi + lo (hi = bf16(x)), keeping hi*hi + lo*hi + hi*lo terms.
    """
    import ml_dtypes

    bf16 = ml_dtypes.bfloat16
    in_maps = []
    for c in range(B):
        p = np.ascontiguousarray(pred[c], dtype=np.float32)
        t = np.ascontiguousarray(target[c], dtype=np.float32)
        psq = (p * p).sum(axis=1)
        tsq = (t * t).sum(axis=1)
        ph = p.astype(bf16)
        pl = (p - ph.astype(np.float32)).astype(bf16)
        th = t.astype(bf16)
        tl = (t - th.astype(np.float32)).astype(bf16)
        psq_h = psq.astype(bf16)
        psq_l = (psq - psq_h.astype(np.float32)).astype(bf16)
        tsq_h = tsq.astype(bf16)
        tsq_l = (tsq - tsq_h.astype(np.float32)).astype(bf16)

        a = np.empty((K_SPLIT, NPTS), dtype=bf16)
        bm = np.empty((K_SPLIT, NPTS), dtype=bf16)
        for d in range(3):
            a[3 * d + 0] = (-2.0 * ph[:, d].astype(np.float32)).astype(bf16)
            a[3 * d + 1] = (-2.0 * pl[:, d].astype(np.float32)).astype(bf16)
            a[3 * d + 2] = a[3 * d + 0]
            bm[3 * d + 0] = th[:, d]
            bm[3 * d + 1] = th[:, d]
            bm[3 * d + 2] = tl[:, d]
        a[9] = psq_h
        a[10] = psq_l
        a[11] = bf16(1.0)
        a[12] = bf16(1.0)
        bm[9] = bf16(1.0)
        bm[10] = bf16(1.0)
        bm[11] = tsq_h
        bm[12] = tsq_l
        in_maps.append({"a": a, "b": bm})
    return in_maps


def finalize(results):
    total = 0.0
    for r in results:
        if "rowmins" in r:
            rowmin = r["rowmins"]  # [128, 32] true row mins
        else:
            rowpart = r["rowpart"].reshape(P, N_TILES, N_GROUPS)
            rowmin = rowpart.min(axis=2)  # [128, 32] true row mins
        colmin = r["colmins"]  # [128, 32] true col mins
        total += rowmin.sum(dtype=np.float64) + colmin.sum(dtype=np.float64)
    return np.asarray(total / (B * NPTS), dtype=np.float32)


def _make_in_maps_split18(pred, target):
    """K=18 bf16 split: full (hi+lo)x(hi+lo) products + 3-term norm splits.
    Reproduces D to ~1e-7 absolute; staging precision then dominates."""
    import ml_dtypes

    bf16 = ml_dtypes.bfloat16
    in_maps = []
    for c in range(B):
        p = np.ascontiguousarray(pred[c], dtype=np.float32)
        t = np.ascontiguousarray(target[c], dtype=np.float32)
        psq = (p.astype(np.float64) ** 2).sum(axis=1).astype(np.float32)
        tsq = (t.astype(np.float64) ** 2).sum(axis=1).astype(np.float32)

        def split2(x):
            h = x.astype(bf16)
            l = (x - h.astype(np.float32)).astype(bf16)
            return h, l

        def split3(x):
            h = x.astype(bf16)
            r = x - h.astype(np.float32)
            m = r.astype(bf16)
            l = (r - m.astype(np.float32)).astype(bf16)
            return h, m, l

        ph, pl = split2(p)
        th, tl = split2(t)
        psq_h, psq_m, psq_l = split3(psq)
        tsq_h, tsq_m, tsq_l = split3(tsq)

        a = np.empty((K_SPLIT2, NPTS), dtype=bf16)
        bm = np.empty((K_SPLIT2, NPTS), dtype=bf16)
        for d in range(3):
            m2h = (-2.0 * ph[:, d].astype(np.float32)).astype(bf16)
            m2l = (-2.0 * pl[:, d].astype(np.float32)).astype(bf16)
            a[4 * d + 0] = m2h
            a[4 * d + 1] = m2l
            a[4 * d + 2] = m2h
            a[4 * d + 3] = m2l
            bm[4 * d + 0] = th[:, d]
            bm[4 * d + 1] = th[:, d]
            bm[4 * d + 2] = tl[:, d]
            bm[4 * d + 3] = tl[:, d]
        a[12], a[13], a[14] = psq_h, psq_m, psq_l
        a[15] = a[16] = a[17] = bf16(1.0)
        bm[12] = bm[13] = bm[14] = bf16(1.0)
        bm[15], bm[16], bm[17] = tsq_h, tsq_m, tsq_l
        in_maps.append({"a": a, "b": bm})
    return in_maps


def kernel(pred, target):
    pred = np.asarray(pred)
    target = np.asarray(target)
    assert pred.shape == (B, NPTS, 3) and target.shape == (B, NPTS, 3)
    return finalize(
        get_runner(variant="v5-b5-x-f16")(
            make_in_maps(pred, target, kind="split18")
        )
    )

